# revision 34
# baseline (speedup 1.0000x reference)
"""Trainium2 Bass kernel: batched FFT along axis 1 of x[64, 4096, 128] (fp32),
returning (real, imag) parts.  8-core data-parallel over the batch axis.

Algorithm (per batch slice [4096, 128]): 4-step Cooley-Tukey with
N = N1*N2 = 128*32, n = 32*n1 + n2, k = 128*k2 + k1:

    X[128*k2 + k1] = sum_n2 T[k1,n2] * W32[n2,k2] * (sum_n1 W128[n1,k1] * x[32*n1+n2])

Phase 1 (per batch):
  - load x as [p=n1, f=n2*128+m]
  - stage-1 DFT-128 over n1 on the PE, rows k1 = 0..64 only (x is real, so
    A[128-k1] = conj(A[k1]); column 64 of the DFT matrix is the Nyquist row)
  - twiddle B = A*exp(-2i*pi*k1*n2/4096) on the DVE (PSUM -> SBUF)
  - write B[65, 4096] to an internal DRAM bounce buffer in n2-major order.
The DRAM round trip performs the k1<->n2 transpose: SBUF<->SBUF DMA cannot
cross partitions, and matmuls cannot target PSUM partitions >= 64 (PE
tiling is broken for 4-byte dtypes), so a [k1 x n2]-packed on-chip
transpose is not expressible.  DRAM APs are unrestricted.

Phase 2 (per batch):
  - load Bd[p=32g+n2, f=jm*128+m] = B[q=32g+jm, n2, m] for g in {0,1}
    (rows k1=0..63), 4KiB-contiguous reads
  - stage-2 DFT-32 over n2 with wide [64,128] stationaries: one matmul
    produces both the direct outputs (k1 = 32g+jm) and the conjugate
    outputs (k1' = 128-q) from the same moving pass, conjugation signs
    folded into the stationary constants.  The Nyquist row feeds a tiny
    separate matmul (its conjugate fixed point needs no sign fixup).
  - ACT evicts PSUM -> SBUF, DMA out in natural k row order.
"""

import numpy as np
from contextlib import ExitStack

import concourse.bacc as bacc
import concourse.bass as bass
import concourse.mybir as mybir
import concourse.tile as tile
from concourse.bass_utils import run_bass_kernel_spmd

N = 4096
N1, N2 = 128, 32
M = 128
B_FULL = 64
NCORES = 8
BPER = B_FULL // NCORES  # 8 batches per core

FP32 = mybir.dt.float32
FP32R = mybir.dt.float32r  # full-rate fp32 matmul streaming format

QROWS = 65           # stored B rows q = 0..64 (Hermitian half + Nyquist)
BD_ROW = QROWS * M   # DRAM bounce stride per n2, in elements


# ---------------------------------------------------------------- constants
def make_consts():
    n1 = np.arange(N1)
    k1 = np.arange(QROWS)
    ang1 = 2 * np.pi * np.outer(n1, k1) / N1
    g_mat = np.cos(ang1).astype(np.float32)              # [128, 65]
    h_mat = (-np.sin(ang1)).astype(np.float32)           # [128, 65]

    n2 = np.arange(N2)
    ang_t = 2 * np.pi * np.outer(k1, n2) / N
    t_re = np.cos(ang_t).astype(np.float32)              # [65, 32] m-bcast
    t_im = (-np.sin(ang_t)).astype(np.float32)

    # stage 2 blocks
    k2v = np.arange(N2)
    a2 = 2 * np.pi * np.outer(n2, k2v) / N2
    a2u = 2 * np.pi * np.outer(n2, k2v + 1) / N2
    w2re = np.cos(a2).astype(np.float32)
    w2im = (-np.sin(a2)).astype(np.float32)
    w2ure = np.cos(a2u).astype(np.float32)
    w2uim = (-np.sin(a2u)).astype(np.float32)

    # wide stationaries [64, 128]: rows p = 32g + n2 (g = rhs group),
    # cols p_out = 32G + k2.  Nonzero blocks:
    #   up (direct):   (g0,G0) k1 = jm      ; (g1,G1) k1 = 32+jm
    #   low (conj):    (g1,G2) k1' = 96-jm  ; (g0,G3) k1' = 128-jm
    def wide(up_blk, low_blk):
        s = np.zeros((64, 128), np.float32)
        s[0:32, 0:32] = up_blk
        s[32:64, 32:64] = up_blk
        s[32:64, 64:96] = low_blk
        s[0:32, 96:128] = low_blk
        return s

    return dict(
        g_mat=g_mat, h_mat=h_mat, t_re=t_re, t_im=t_im,
        su_a=wide(w2re, w2ure),      # C_re <- Bd_re
        su_b=wide(-w2im, w2uim),     # C_re <- Bd_im
        su_c=wide(w2im, w2uim),      # C_im <- Bd_re
        su_d=wide(w2re, -w2ure),     # C_im <- Bd_im
        w2re=w2re, w2im=w2im, nw2im=(-w2im).astype(np.float32).copy(),
    )


CONST_SHAPES = {
    "g_mat": (128, 65), "h_mat": (128, 65),
    "t_re": (65, 32), "t_im": (65, 32),
    "su_a": (64, 128), "su_b": (64, 128), "su_c": (64, 128), "su_d": (64, 128),
    "w2re": (32, 32), "w2im": (32, 32), "nw2im": (32, 32),
}

# constants that feed the PE as stationaries use the f32r streaming format
MM_CONSTS = {"g_mat", "h_mat", "su_a", "su_b", "su_c", "su_d",
             "w2re", "w2im", "nw2im"}


def _hand_ap(base_ap, rel_off, dims):
    return bass.AP(tensor=base_ap.tensor, offset=base_ap.offset + rel_off,
                   ap=[list(d) for d in dims])


# ---------------------------------------------------------------- program
def build_program(use_f32r=True):
    nc = bacc.Bacc("TRN2", target_bir_lowering=False, debug=False)

    MMDT = FP32R if use_f32r else FP32
    x_in = nc.dram_tensor("x", [BPER, N, M], MMDT, kind="ExternalInput")
    out_re = nc.dram_tensor("out_re", [BPER, N, M], FP32, kind="ExternalOutput")
    out_im = nc.dram_tensor("out_im", [BPER, N, M], FP32, kind="ExternalOutput")
    cin = {k: nc.dram_tensor(k, list(v), MMDT if k in MM_CONSTS else FP32,
                             kind="ExternalInput")
           for k, v in CONST_SHAPES.items()}
    # DRAM bounce: [b][n2][q][m], n2-major so phase-2 reads are 4KiB runs
    bdram = {c: nc.dram_tensor(f"bdram_{c}", [BPER, N2, QROWS, M], MMDT,
                               kind="Internal")
             for c in ("re", "im")}

    with tile.TileContext(nc) as tc, ExitStack() as ctx:
        cpool = ctx.enter_context(tc.tile_pool(name="consts", bufs=1))
        ct = {}
        for k, shp in CONST_SHAPES.items():
            ct[k] = cpool.tile(list(shp), MMDT if k in MM_CONSTS else FP32,
                               tag=k, name=f"ct_{k}")
            nc.sync.dma_start(ct[k][:], cin[k].ap())

        x_pool = ctx.enter_context(tc.tile_pool(name="x", bufs=2))
        a_psum = ctx.enter_context(tc.tile_pool(name="apsum", bufs=1, space="PSUM"))
        c_psum = ctx.enter_context(tc.tile_pool(name="cpsum", bufs=2, space="PSUM"))
        tw_pool = ctx.enter_context(tc.tile_pool(name="tw", bufs=1))
        b_pool = ctx.enter_context(tc.tile_pool(name="b", bufs=2))
        nyq_pool = ctx.enter_context(tc.tile_pool(name="nyq", bufs=2))
        bd_pool = ctx.enter_context(tc.tile_pool(name="bd", bufs=2))
        cs_pool = ctx.enter_context(tc.tile_pool(name="cs", bufs=2))
        cs64_pool = ctx.enter_context(tc.tile_pool(name="cs64", bufs=2))

        # ================= phase 1: stage-1 + twiddle + bounce write ======
        def phase1(b):
            X = x_pool.tile([128, N2 * M], MMDT, tag="X")
            nc.sync.dma_start(X[:], x_in.ap()[b])

            b_re_t = b_pool.tile([QROWS, N2 * M], MMDT, tag="b_re")
            b_im_t = b_pool.tile([QROWS, N2 * M], MMDT, tag="b_im")
            b_re, b_im = b_re_t[:], b_im_t[:]

            for fg in range(4):
                fs = slice(1024 * fg, 1024 * fg + 1024)
                a_re = a_psum.tile([QROWS, 1024], FP32, tag="a_re")
                a_im = a_psum.tile([QROWS, 1024], FP32, tag="a_im")
                for cc in (0, 1):
                    cw = slice(512 * cc, 512 * cc + 512)
                    rhs = X[:, 1024 * fg + 512 * cc: 1024 * fg + 512 * cc + 512]
                    nc.tensor.matmul(a_re[:, cw], ct["g_mat"][:], rhs,
                                     start=True, stop=True)
                    nc.tensor.matmul(a_im[:, cw], ct["h_mat"][:], rhs,
                                     start=True, stop=True)

                # compact twiddle [65, 8] broadcast over m via step-0 AP
                n2s = slice(8 * fg, 8 * fg + 8)
                tre_b = ct["t_re"][:, n2s].unsqueeze(2).broadcast_to(
                    [QROWS, 8, 128])
                tim_b = ct["t_im"][:, n2s].unsqueeze(2).broadcast_to(
                    [QROWS, 8, 128])
                a_re3 = a_re[:].rearrange("p (a m) -> p a m", m=128)
                a_im3 = a_im[:].rearrange("p (a m) -> p a m", m=128)
                p1 = tw_pool.tile([QROWS, 1024], FP32, tag="p1")
                p2 = tw_pool.tile([QROWS, 1024], FP32, tag="p2")
                p13 = p1[:].rearrange("p (a m) -> p a m", m=128)
                p23 = p2[:].rearrange("p (a m) -> p a m", m=128)
                nc.vector.tensor_mul(p13, a_re3, tre_b)
                nc.vector.tensor_mul(p23, a_im3, tim_b)
                nc.gpsimd.tensor_sub(b_re[:, fs], p1[:], p2[:])
                p3 = tw_pool.tile([QROWS, 1024], FP32, tag="p3")
                p4 = tw_pool.tile([QROWS, 1024], FP32, tag="p4")
                p33 = p3[:].rearrange("p (a m) -> p a m", m=128)
                p43 = p4[:].rearrange("p (a m) -> p a m", m=128)
                nc.vector.tensor_mul(p33, a_re3, tim_b)
                nc.vector.tensor_mul(p43, a_im3, tre_b)
                nc.gpsimd.tensor_add(b_im[:, fs], p3[:], p4[:])

            # bounce write: B[p = k1, f = n2*128+m] -> bdram[b, n2, q=k1, m]
            for comp, bsb in (("re", b_re), ("im", b_im)):
                dst = _hand_ap(bdram[comp].ap(), b * N2 * BD_ROW,
                               [[M, QROWS], [BD_ROW, N2], [1, M]])
                nc.sync.dma_start(dst, bsb)

        # ================= phase 2: load + stage-2 + out ==================
        def phase2(b):
            bd_re = bd_pool.tile([64, 4096], MMDT, tag="bd_re")
            bd_im = bd_pool.tile([64, 4096], MMDT, tag="bd_im")
            n32_re = nyq_pool.tile([32, 128], MMDT, tag="n32_re")
            n32_im = nyq_pool.tile([32, 128], MMDT, tag="n32_im")
            for comp, bd, n32 in (("re", bd_re, n32_re), ("im", bd_im, n32_im)):
                dram = bdram[comp].ap()
                for g in (0, 1):
                    src = _hand_ap(dram, b * N2 * BD_ROW + 32 * g * M,
                                   [[BD_ROW, 32], [1, 32 * M]])
                    nc.scalar.dma_start(bd[:][32 * g:32 * g + 32, :], src)
                src32 = _hand_ap(dram, b * N2 * BD_ROW + 64 * M,
                                 [[BD_ROW, 32], [1, M]])
                nc.scalar.dma_start(n32[:], src32)

            # Nyquist outputs: rows k = 128*k2 + 64 (borrows a cpsum slot)
            c64 = c_psum.tile([128, 512], FP32, tag="c_re")
            nc.tensor.matmul(c64[0:32, 0:128], ct["w2re"][:], n32_re[:],
                             start=True, stop=False)
            nc.tensor.matmul(c64[0:32, 0:128], ct["nw2im"][:], n32_im[:],
                             start=False, stop=True)
            nc.tensor.matmul(c64[0:32, 128:256], ct["w2im"][:], n32_re[:],
                             start=True, stop=False)
            nc.tensor.matmul(c64[0:32, 128:256], ct["w2re"][:], n32_im[:],
                             start=False, stop=True)
            cs64_re = cs64_pool.tile([32, 128], FP32, tag="cs64_re")
            cs64_im = cs64_pool.tile([32, 128], FP32, tag="cs64_im")
            nc.scalar.copy(cs64_re[:], c64[0:32, 0:128])
            nc.scalar.copy(cs64_im[:], c64[0:32, 128:256])
            for dram_t, cs_t in ((out_re, cs64_re), (out_im, cs64_im)):
                dst = _hand_ap(dram_t.ap(), b * N * M + 64 * M,
                               [[128 * M, 32], [1, M]])
                nc.sync.dma_start(dst, cs_t[:])

            # main stage 2, per quarter (jm = 8h + jml, jml = 0..7)
            for h in range(4):
                cs_re = cs_pool.tile([128, 1024], FP32, tag="cs_re")
                cs_im = cs_pool.tile([128, 1024], FP32, tag="cs_im")
                # group matmuls by stationary across the chunk pair to
                # halve PE weight reloads; psum pairs use the 2 pool slots
                cres, cims, rres, rims = [], [], [], []
                for cc in range(2):
                    ch = slice(512 * (2 * h + cc), 512 * (2 * h + cc) + 512)
                    rres.append(bd_re[:][:, ch])
                    rims.append(bd_im[:][:, ch])
                    cres.append(c_psum.tile([128, 512], FP32, tag="c_re",
                                            name=f"c_re_{h}_{cc}"))
                    cims.append(c_psum.tile([128, 512], FP32, tag="c_im",
                                            name=f"c_im_{h}_{cc}"))
                for cc in range(2):
                    nc.tensor.matmul(cres[cc][:], ct["su_a"][:], rres[cc],
                                     start=True, stop=False)
                for cc in range(2):
                    nc.tensor.matmul(cres[cc][:], ct["su_b"][:], rims[cc],
                                     start=False, stop=True)
                for cc in range(2):
                    nc.tensor.matmul(cims[cc][:], ct["su_c"][:], rres[cc],
                                     start=True, stop=False)
                for cc in range(2):
                    nc.tensor.matmul(cims[cc][:], ct["su_d"][:], rims[cc],
                                     start=False, stop=True)
                for cc in range(2):
                    cw = slice(512 * cc, 512 * cc + 512)
                    nc.scalar.copy(cs_re[:, cw], cres[cc][:])
                    nc.scalar.copy(cs_im[:, cw], cims[cc][:])

                # out rows: p = 32G + k2, f = jml*128 + m
                #   G0: 128k2 + jm      G1: 128k2 + 32 + jm
                #   G2: 128k2 + 96 - jm G3: 128k2 + 128 - jm (jm=0 dead)
                for dram_t, cs_t in ((out_re, cs_re), (out_im, cs_im)):
                    dap = dram_t.ap()
                    base = b * N * M
                    dst = _hand_ap(dap, base + 1024 * h,
                                   [[32 * M, 2], [128 * M, 32], [1, 1024]])
                    nc.sync.dma_start(dst, cs_t[0:64, :])
                    dst = _hand_ap(dap, base + 96 * M - 1024 * h,
                                   [[128 * M, 32], [-M, 8], [1, M]])
                    nc.sync.dma_start(dst, cs_t[64:96, :])
                    if h == 0:
                        dst = _hand_ap(dap, base + 128 * M - M,
                                       [[128 * M, 32], [-M, 7], [1, M]])
                        nc.sync.dma_start(dst, cs_t[96:128, M:])
                    else:
                        dst = _hand_ap(dap, base + 128 * M - 1024 * h,
                                       [[128 * M, 32], [-M, 8], [1, M]])
                        nc.sync.dma_start(dst, cs_t[96:128, :])

        # interleave with a lag so phase-2(b) overlaps phase-1(b+LAG)
        LAG = BPER  # sequential phases scheduled best
        for b in range(BPER + LAG):
            if b < BPER:
                phase1(b)
            if b >= LAG:
                phase2(b - LAG)

    nc.compile()
    return nc


_CACHE = {}


def _get_program():
    if "nc" not in _CACHE:
        _CACHE["nc"] = build_program()
        _CACHE["consts"] = make_consts()
    return _CACHE["nc"], _CACHE["consts"]


_LAST = {}


def _run(x: np.ndarray, trace: bool = False):
    x = np.ascontiguousarray(np.asarray(x, dtype=np.float32))
    assert x.shape == (B_FULL, N, M)
    nc, consts = _get_program()
    in_maps = []
    for c in range(NCORES):
        m = {"x": np.ascontiguousarray(x[c * BPER:(c + 1) * BPER])}
        m.update(consts)
        in_maps.append(m)
    bres = run_bass_kernel_spmd(nc, in_maps, list(range(NCORES)), trace=trace)
    _LAST["results"] = bres
    res = bres.results
    re = np.concatenate([res[c]["out_re"] for c in range(NCORES)], axis=0)
    im = np.concatenate([res[c]["out_im"] for c in range(NCORES)], axis=0)
    return re, im


def kernel(x: np.ndarray):
    """x: [64, 4096, 128] fp32 -> (re, im) each [64, 4096, 128] fp32."""
    return _run(x, trace=False)
